# revision 5
# baseline (speedup 1.0000x reference)
"""Dense transformer block (LN+MHSA+residual, LN+GELU-MLP+residual) on 8 TRN2
NeuronCores. Data-parallel: core c handles batch c//2, query-half c%2 (1024
tokens). Each core recomputes K/V for its full batch (2048 tokens) from a
token-rolled feature-major copy of x, so no collectives are needed.

All matmuls run in bf16 (fp32 PSUM accumulation); LayerNorm statistics are
computed with ones-matmuls on the feature-major activations so no on-chip
transposes are ever needed.
"""
import sys

for _p in ("/opt/trn_rl_repo", "/root/.axon_site/_ro/trn_rl_repo"):
    if _p not in sys.path:
        sys.path.insert(0, _p)

import numpy as np
import ml_dtypes

import concourse.bass as bass
import concourse.bacc as bacc
import concourse.tile as tile
from concourse import mybir
from concourse.bass_utils import run_bass_kernel_spmd

f32 = mybir.dt.float32
bf16 = mybir.dt.bfloat16
AF = mybir.ActivationFunctionType
Alu = mybir.AluOpType

B, T, D = 4, 2048, 512
H = 8
DQ = D // H          # 64
MLP = 2048
TOK = 1024           # query tokens per core
EPS = 1e-5
SCALE = DQ ** -0.5   # 0.125

_built = None


def _build():
    nc = bacc.Bacc()

    xtok_d = nc.dram_tensor("x_tok", [TOK, D], f32, kind="ExternalInput")
    xT_d = nc.dram_tensor("xT", [D, T], bf16, kind="ExternalInput")
    wq_d = nc.dram_tensor("wq", [D, D], bf16, kind="ExternalInput")
    wk_d = nc.dram_tensor("wk", [D, D], bf16, kind="ExternalInput")
    wv_d = nc.dram_tensor("wv", [D, D], bf16, kind="ExternalInput")
    wo_d = nc.dram_tensor("wo", [D, D], bf16, kind="ExternalInput")
    w1_d = nc.dram_tensor("w1", [D, MLP], bf16, kind="ExternalInput")
    w2_d = nc.dram_tensor("w2", [MLP, D], bf16, kind="ExternalInput")
    out_d = nc.dram_tensor("out", [TOK, D], f32, kind="ExternalOutput")

    from contextlib import ExitStack

    with tile.TileContext(nc) as tc:
        with ExitStack() as ctx:
            sb = ctx.enter_context(tc.tile_pool(name="sb", bufs=1))
            xtp = ctx.enter_context(tc.tile_pool(name="xtp", bufs=4))
            sqp = ctx.enter_context(tc.tile_pool(name="sqs", bufs=2))
            statp = ctx.enter_context(tc.tile_pool(name="stat", bufs=2))
            statfp = ctx.enter_context(tc.tile_pool(name="statf", bufs=3))
            wsp = ctx.enter_context(tc.tile_pool(name="wsheet", bufs=4))
            shp = ctx.enter_context(tc.tile_pool(name="sheet", bufs=4))
            ktp = ctx.enter_context(tc.tile_pool(name="kt", bufs=4))
            htp = ctx.enter_context(tc.tile_pool(name="ht", bufs=9))
            qtp = ctx.enter_context(tc.tile_pool(name="qt", bufs=4))
            vp = ctx.enter_context(tc.tile_pool(name="v512", bufs=16))
            ep = ctx.enter_context(tc.tile_pool(name="ep", bufs=2))
            attnp = ctx.enter_context(tc.tile_pool(name="attn", bufs=8))
            tokp = ctx.enter_context(tc.tile_pool(name="tok", bufs=9))
            ytp = ctx.enter_context(tc.tile_pool(name="yt", bufs=8))
            zrp = ctx.enter_context(tc.tile_pool(name="zr", bufs=2))
            ps2 = ctx.enter_context(tc.tile_pool(name="ps2", bufs=3, space="PSUM"))
            ps1 = ctx.enter_context(tc.tile_pool(name="ps1", bufs=2, space="PSUM"))
            # ---------------- constants ----------------
            ones128 = sb.tile([128, 128], bf16)
            nc.vector.memset(ones128, 1.0)
            eps_t = sb.tile([128, 1], f32)
            nc.vector.memset(eps_t, EPS)

            # ---------------- input DMAs ----------------
            xt = []
            for dc in range(4):
                t = xtp.tile([128, T], bf16, tag="xt")
                nc.sync.dma_start(out=t, in_=xT_d[dc * 128:(dc + 1) * 128, :])
                xt.append(t)

            xtok_sb = []
            for tt in range(8):
                t = tokp.tile([128, D], f32, tag="tok")
                nc.sync.dma_start(out=t, in_=xtok_d[tt * 128:(tt + 1) * 128, :])
                xtok_sb.append(t)

            # wq/wk/wv as [128, 4(dc), 512(j)]
            def load_w4(dram):
                t = wsp.tile([128, 4, 512], bf16, tag="w")
                nc.sync.dma_start(
                    out=t, in_=dram.ap().rearrange("(a p) j -> p a j", p=128))
                return t

            wq_sb = load_w4(wq_d)
            wk_sb = load_w4(wk_d)
            wv_sb = load_w4(wv_d)

            wo_sb = sb.tile([64, 8, 512], bf16)   # [dq, head, j]
            nc.sync.dma_start(
                out=wo_sb, in_=wo_d.ap().rearrange("(h p) j -> p h j", p=64))

            w1_sb = []
            for jc in range(4):
                t = wsp.tile([128, 2048], bf16, tag="w")
                nc.sync.dma_start(out=t, in_=w1_d[jc * 128:(jc + 1) * 128, :])
                w1_sb.append(t)

            # ---- LN stats from feature-major bf16 chunks (ones-matmuls) ----
            def ln_stats(src_tiles, tlen):
                mu_bf = statp.tile([128, 2048], bf16, tag="stat")
                rstd_bf = statp.tile([128, 2048], bf16, tag="stat")
                for tch in range(tlen // 1024):
                    sl = slice(tch * 1024, (tch + 1) * 1024)
                    mu_ps = ps2.tile([128, 1024], f32, tag="ps2")
                    sq_ps = ps2.tile([128, 1024], f32, tag="ps2")
                    for half in range(2):
                        hs = slice(tch * 1024 + half * 512,
                                   tch * 1024 + half * 512 + 512)
                        ps_h = slice(half * 512, half * 512 + 512)
                        for dc in range(4):
                            nc.tensor.matmul(mu_ps[:, ps_h], lhsT=ones128,
                                             rhs=src_tiles[dc][:, hs],
                                             start=(dc == 0), stop=(dc == 3))
                        for dc in range(4):
                            s = sqp.tile([128, 512], bf16, tag="sq")
                            nc.vector.tensor_mul(out=s, in0=src_tiles[dc][:, hs],
                                                 in1=src_tiles[dc][:, hs])
                            nc.tensor.matmul(sq_ps[:, ps_h], lhsT=ones128, rhs=s,
                                             start=(dc == 0), stop=(dc == 3))
                    mu_f = statfp.tile([128, 1024], f32, tag="statf")
                    var_f = statfp.tile([128, 1024], f32, tag="statf")
                    tmp = statfp.tile([128, 1024], f32, tag="statf")
                    nc.vector.tensor_scalar_mul(out=mu_f, in0=mu_ps, scalar1=1.0 / D)
                    nc.vector.tensor_scalar_mul(out=var_f, in0=sq_ps, scalar1=1.0 / D)
                    nc.vector.tensor_mul(out=tmp, in0=mu_f, in1=mu_f)
                    nc.vector.tensor_sub(out=var_f, in0=var_f, in1=tmp)
                    # rstd = exp(-0.5 * ln(var + eps))
                    nc.scalar.activation(out=var_f, in_=var_f, func=AF.Ln,
                                         bias=eps_t[:, :])
                    nc.scalar.activation(out=rstd_bf[:, sl], in_=var_f,
                                         func=AF.Exp, scale=-0.5)
                    nc.vector.tensor_copy(out=mu_bf[:, sl], in_=mu_f)
                return mu_bf, rstd_bf

            def ln_apply(src_tiles, mu_bf, rstd_bf, tlen, tag, pool):
                outs = []
                for dc in range(4):
                    o = pool.tile([128, tlen], bf16, tag=tag)
                    nc.vector.tensor_sub(out=o, in0=src_tiles[dc],
                                         in1=mu_bf[:, 0:tlen])
                    nc.vector.tensor_mul(out=o, in0=o, in1=rstd_bf[:, 0:tlen])
                    outs.append(o)
                return outs

            # ================= stage 1: LN1 -> xnT =================
            mu1, rstd1 = ln_stats(xt, T)
            xnT = ln_apply(xt, mu1, rstd1, T, "sheet", shp)

            # ================= QKV =================
            qT = []
            for jc in range(4):
                q_ps = ps2.tile([128, 1024], f32, tag="ps2")
                for half in range(2):
                    ps_h = slice(half * 512, half * 512 + 512)
                    for dc in range(4):
                        nc.tensor.matmul(
                            q_ps[:, ps_h],
                            lhsT=wq_sb[:, dc, jc * 128:jc * 128 + 128],
                            rhs=xnT[dc][:, half * 512:half * 512 + 512],
                            start=(dc == 0), stop=(dc == 3))
                q_t = qtp.tile([128, 1024], bf16, tag="qt")
                nc.vector.tensor_copy(out=q_t, in_=q_ps)
                qT.append(q_t)

            # V token-major (form 1) — before K so xnT frees for kT slots
            v_sb = []
            for tt in range(16):
                v_ps = ps1.tile([128, 512], f32, tag="ps1")
                for dc in range(4):
                    nc.tensor.matmul(
                        v_ps,
                        lhsT=xnT[dc][:, tt * 128:(tt + 1) * 128],
                        rhs=wv_sb[:, dc, :],
                        start=(dc == 0), stop=(dc == 3))
                v_t = vp.tile([128, 512], bf16, tag="v")
                nc.vector.tensor_copy(out=v_t, in_=v_ps)
                v_sb.append(v_t)

            kT = []
            for jc in range(4):
                k_t = ktp.tile([128, T], bf16, tag="kt")
                for tch in range(2):
                    k_ps = ps2.tile([128, 1024], f32, tag="ps2")
                    for half in range(2):
                        ts0 = tch * 1024 + half * 512
                        ps_h = slice(half * 512, half * 512 + 512)
                        for dc in range(4):
                            nc.tensor.matmul(
                                k_ps[:, ps_h],
                                lhsT=wk_sb[:, dc, jc * 128:jc * 128 + 128],
                                rhs=xnT[dc][:, ts0:ts0 + 512],
                                start=(dc == 0), stop=(dc == 3))
                    nc.vector.tensor_copy(
                        out=k_t[:, tch * 1024:(tch + 1) * 1024], in_=k_ps)
                kT.append(k_t)

            # w2 DMAs now: reuse v-tag slots as they free after attention
            w2_sb = []
            for mc in range(16):
                t = vp.tile([128, 512], bf16, tag="v")
                nc.sync.dma_start(out=t, in_=w2_d[mc * 128:(mc + 1) * 128, :])
                w2_sb.append(t)

            # ================= attention =================
            attn_h = []
            for _h in range(H):
                a_t = attnp.tile([64, 1024], bf16, tag="attn")
                attn_h.append(a_t)
            for h8 in range(H):
                jc = h8 // 2
                rb = (h8 % 2) * 64
                for qc in range(2):
                    o_ps = ps1.tile([64, 512], f32, tag="ps1")
                    z_ps = ps1.tile([64, 512], f32, tag="ps1")
                    for kcp in range(8):
                        s_ps = ps2.tile([128, 1024], f32, tag="ps2")
                        for j in range(2):
                            kc = kcp * 2 + j
                            nc.tensor.matmul(
                                s_ps[:, j * 512:(j + 1) * 512],
                                lhsT=kT[jc][rb:rb + 64, kc * 128:(kc + 1) * 128],
                                rhs=qT[jc][rb:rb + 64, qc * 512:(qc + 1) * 512],
                                start=True, stop=True)
                        e_t = ep.tile([128, 1024], bf16, tag="e")
                        nc.scalar.activation(out=e_t, in_=s_ps, func=AF.Exp,
                                             scale=SCALE)
                        for j in range(2):
                            kc = kcp * 2 + j
                            sl = slice(j * 512, (j + 1) * 512)
                            nc.tensor.matmul(
                                o_ps, lhsT=v_sb[kc][:, h8 * 64:h8 * 64 + 64],
                                rhs=e_t[:, sl],
                                start=(kc == 0), stop=(kc == 15))
                            nc.tensor.matmul(
                                z_ps, lhsT=ones128[:, 0:64], rhs=e_t[:, sl],
                                start=(kc == 0), stop=(kc == 15))
                    z_sb = zrp.tile([64, 512], f32, tag="z")
                    nc.vector.tensor_copy(out=z_sb, in_=z_ps)
                    r_t = zrp.tile([64, 512], f32, tag="r")
                    nc.vector.reciprocal_approx_fast(out=r_t, in_=z_sb)
                    nc.vector.tensor_mul(
                        out=attn_h[h8][:, qc * 512:(qc + 1) * 512],
                        in0=o_ps, in1=r_t)

            # ================= O-projection + residuals =================
            y_sb = []
            for tt in range(8):
                p_ps = ps1.tile([128, 512], f32, tag="ps1")
                for h8 in range(H):
                    nc.tensor.matmul(
                        p_ps, lhsT=attn_h[h8][:, tt * 128:(tt + 1) * 128],
                        rhs=wo_sb[:, h8, :],
                        start=(h8 == 0), stop=(h8 == 7))
                y_t = tokp.tile([128, D], f32, tag="tok")
                nc.vector.tensor_add(out=y_t, in0=p_ps, in1=xtok_sb[tt])
                y_sb.append(y_t)

            yT = []
            for jc in range(4):
                p_ps = ps2.tile([128, 1024], f32, tag="ps2")
                for half in range(2):
                    ps_h = slice(half * 512, half * 512 + 512)
                    for h8 in range(H):
                        nc.tensor.matmul(
                            p_ps[:, ps_h],
                            lhsT=wo_sb[:, h8, jc * 128:jc * 128 + 128],
                            rhs=attn_h[h8][:, half * 512:half * 512 + 512],
                            start=(h8 == 0), stop=(h8 == 7))
                y_t = ytp.tile([128, 1024], bf16, tag="yt")
                nc.vector.tensor_add(out=y_t, in0=p_ps, in1=xt[jc][:, 0:1024])
                yT.append(y_t)

            # ================= LN2 -> ynT =================
            mu2, rstd2 = ln_stats(yT, 1024)
            ynT = ln_apply(yT, mu2, rstd2, 1024, "yt", ytp)

            # ================= MLP =================
            def mlp1(mc):
                h_ps = ps2.tile([128, 1024], f32, tag="ps2")
                for half in range(2):
                    ps_h = slice(half * 512, half * 512 + 512)
                    for jc in range(4):
                        nc.tensor.matmul(
                            h_ps[:, ps_h],
                            lhsT=w1_sb[jc][:, mc * 128:(mc + 1) * 128],
                            rhs=ynT[jc][:, half * 512:half * 512 + 512],
                            start=(jc == 0), stop=(jc == 3))
                h_t = htp.tile([128, 1024], bf16, tag="ht")
                nc.scalar.activation(out=h_t, in_=h_ps, func=AF.Gelu)
                return h_t

            hT = [mlp1(mc) for mc in range(8)]
            # pass A: mc 0..7
            y2 = []
            for tt in range(8):
                o_ps = ps1.tile([128, 512], f32, tag="ps1")
                for mc in range(8):
                    nc.tensor.matmul(
                        o_ps, lhsT=hT[mc][:, tt * 128:(tt + 1) * 128],
                        rhs=w2_sb[mc],
                        start=(mc == 0), stop=(mc == 7))
                y_t = tokp.tile([128, D], f32, tag="tok")
                nc.vector.tensor_add(out=y_t, in0=o_ps, in1=y_sb[tt])
                y2.append(y_t)
            hTb = [mlp1(mc) for mc in range(8, 16)]
            for tt in range(8):
                o_ps = ps1.tile([128, 512], f32, tag="ps1")
                for mc in range(8):
                    nc.tensor.matmul(
                        o_ps, lhsT=hTb[mc][:, tt * 128:(tt + 1) * 128],
                        rhs=w2_sb[8 + mc],
                        start=(mc == 0), stop=(mc == 7))
                y_t = tokp.tile([128, D], f32, tag="tok")
                nc.vector.tensor_add(out=y_t, in0=o_ps, in1=y2[tt])
                nc.sync.dma_start(out=out_d[tt * 128:(tt + 1) * 128, :], in_=y_t)

    nc.compile()
    return nc


def kernel(**inputs):
    global _built
    x = np.asarray(inputs["x"], dtype=np.float32)
    wbf = {n: np.ascontiguousarray(
        np.asarray(inputs[n], dtype=np.float32).astype(ml_dtypes.bfloat16))
        for n in ("Wq", "Wk", "Wv", "Wo", "W1", "W2")}

    if _built is None:
        _built = _build()
    nc = _built

    in_maps = []
    for c in range(8):
        b, hh = c // 2, c % 2
        own = x[b, hh * TOK:(hh + 1) * TOK]
        other = x[b, (1 - hh) * TOK:(2 - hh) * TOK]
        roll = np.concatenate([own, other], axis=0)           # [2048, 512]
        xT = np.ascontiguousarray(roll.T.astype(ml_dtypes.bfloat16))
        in_maps.append({
            "x_tok": np.ascontiguousarray(own),
            "xT": xT,
            "wq": wbf["Wq"], "wk": wbf["Wk"], "wv": wbf["Wv"], "wo": wbf["Wo"],
            "w1": wbf["W1"], "w2": wbf["W2"],
        })

    res = run_bass_kernel_spmd(nc, in_maps, core_ids=list(range(8)))
    out = np.empty((B, T, D), np.float32)
    for c in range(8):
        b, hh = c // 2, c % 2
        out[b, hh * TOK:(hh + 1) * TOK] = res.results[c]["out"]
    return out


# revision 6
# speedup vs baseline: 1.0241x; 1.0241x over previous
"""Dense transformer block (LN+MHSA+residual, LN+GELU-MLP+residual) on 8 TRN2
NeuronCores. Data-parallel: core c handles batch c//2, query-half c%2 (1024
tokens). Each core recomputes K/V for its full batch (2048 tokens) from a
token-rolled feature-major copy of x, so no collectives are needed.

All matmuls run in bf16 (fp32 PSUM accumulation); LayerNorm statistics are
computed with ones-matmuls on the feature-major activations so no on-chip
transposes are ever needed.
"""
import sys

for _p in ("/opt/trn_rl_repo", "/root/.axon_site/_ro/trn_rl_repo"):
    if _p not in sys.path:
        sys.path.insert(0, _p)

import numpy as np
import ml_dtypes

import concourse.bass as bass
import concourse.bacc as bacc
import concourse.tile as tile
from concourse import mybir
from concourse.bass_utils import run_bass_kernel_spmd

f32 = mybir.dt.float32
bf16 = mybir.dt.bfloat16
AF = mybir.ActivationFunctionType
Alu = mybir.AluOpType

B, T, D = 4, 2048, 512
H = 8
DQ = D // H          # 64
MLP = 2048
TOK = 1024           # query tokens per core
EPS = 1e-5
SCALE = DQ ** -0.5   # 0.125

_built = None


def _build():
    nc = bacc.Bacc()

    xtok_d = nc.dram_tensor("x_tok", [TOK, D], f32, kind="ExternalInput")
    xT_d = nc.dram_tensor("xT", [D, T], bf16, kind="ExternalInput")
    wq_d = nc.dram_tensor("wq", [D, D], bf16, kind="ExternalInput")
    wk_d = nc.dram_tensor("wk", [D, D], bf16, kind="ExternalInput")
    wv_d = nc.dram_tensor("wv", [D, D], bf16, kind="ExternalInput")
    wo_d = nc.dram_tensor("wo", [D, D], bf16, kind="ExternalInput")
    w1_d = nc.dram_tensor("w1", [D, MLP], bf16, kind="ExternalInput")
    w2_d = nc.dram_tensor("w2", [MLP, D], bf16, kind="ExternalInput")
    out_d = nc.dram_tensor("out", [TOK, D], f32, kind="ExternalOutput")

    from contextlib import ExitStack

    with tile.TileContext(nc) as tc:
        with ExitStack() as ctx:
            sb = ctx.enter_context(tc.tile_pool(name="sb", bufs=1))
            xtp = ctx.enter_context(tc.tile_pool(name="xtp", bufs=4))
            sqp = ctx.enter_context(tc.tile_pool(name="sqs", bufs=2))
            statp = ctx.enter_context(tc.tile_pool(name="stat", bufs=2))
            statfp = ctx.enter_context(tc.tile_pool(name="statf", bufs=3))
            wsp = ctx.enter_context(tc.tile_pool(name="wsheet", bufs=4))
            shp = ctx.enter_context(tc.tile_pool(name="sheet", bufs=4))
            ktp = ctx.enter_context(tc.tile_pool(name="kt", bufs=4))
            htp = ctx.enter_context(tc.tile_pool(name="ht", bufs=9))
            qtp = ctx.enter_context(tc.tile_pool(name="qt", bufs=4))
            vp = ctx.enter_context(tc.tile_pool(name="v512", bufs=16))
            ep = ctx.enter_context(tc.tile_pool(name="ep", bufs=2))
            attnp = ctx.enter_context(tc.tile_pool(name="attn", bufs=8))
            tokp = ctx.enter_context(tc.tile_pool(name="tok", bufs=9))
            ytp = ctx.enter_context(tc.tile_pool(name="yt", bufs=8))
            zrp = ctx.enter_context(tc.tile_pool(name="zr", bufs=4))
            ps2 = ctx.enter_context(tc.tile_pool(name="ps2", bufs=2, space="PSUM"))
            ps1 = ctx.enter_context(tc.tile_pool(name="ps1", bufs=4, space="PSUM"))
            # ---------------- constants ----------------
            ones128 = sb.tile([128, 128], bf16)
            nc.vector.memset(ones128, 1.0)
            eps_t = sb.tile([128, 1], f32)
            nc.vector.memset(eps_t, EPS)

            # ---------------- input DMAs ----------------
            xt = []
            for dc in range(4):
                t = xtp.tile([128, T], bf16, tag="xt")
                nc.sync.dma_start(out=t, in_=xT_d[dc * 128:(dc + 1) * 128, :])
                xt.append(t)

            xtok_sb = []
            for tt in range(8):
                t = tokp.tile([128, D], f32, tag="tok")
                nc.sync.dma_start(out=t, in_=xtok_d[tt * 128:(tt + 1) * 128, :])
                xtok_sb.append(t)

            # wq/wk/wv as [128, 4(dc), 512(j)]
            def load_w4(dram):
                t = wsp.tile([128, 4, 512], bf16, tag="w")
                nc.sync.dma_start(
                    out=t, in_=dram.ap().rearrange("(a p) j -> p a j", p=128))
                return t

            wq_sb = load_w4(wq_d)
            wk_sb = load_w4(wk_d)
            wv_sb = load_w4(wv_d)

            wo_sb = sb.tile([64, 8, 512], bf16)   # [dq, head, j]
            nc.sync.dma_start(
                out=wo_sb, in_=wo_d.ap().rearrange("(h p) j -> p h j", p=64))

            w1_sb = []
            for jc in range(4):
                t = wsp.tile([128, 2048], bf16, tag="w")
                nc.sync.dma_start(out=t, in_=w1_d[jc * 128:(jc + 1) * 128, :])
                w1_sb.append(t)

            # ---- LN stats from feature-major bf16 chunks (ones-matmuls) ----
            def ln_stats(src_tiles, tlen):
                mu_bf = statp.tile([128, 2048], bf16, tag="stat")
                rstd_bf = statp.tile([128, 2048], bf16, tag="stat")
                for tch in range(tlen // 1024):
                    sl = slice(tch * 1024, (tch + 1) * 1024)
                    mu_ps = ps2.tile([128, 1024], f32, tag="ps2")
                    sq_ps = ps2.tile([128, 1024], f32, tag="ps2")
                    for half in range(2):
                        hs = slice(tch * 1024 + half * 512,
                                   tch * 1024 + half * 512 + 512)
                        ps_h = slice(half * 512, half * 512 + 512)
                        for dc in range(4):
                            nc.tensor.matmul(mu_ps[:, ps_h], lhsT=ones128,
                                             rhs=src_tiles[dc][:, hs],
                                             start=(dc == 0), stop=(dc == 3))
                        for dc in range(4):
                            s = sqp.tile([128, 512], bf16, tag="sq")
                            nc.vector.tensor_mul(out=s, in0=src_tiles[dc][:, hs],
                                                 in1=src_tiles[dc][:, hs])
                            nc.tensor.matmul(sq_ps[:, ps_h], lhsT=ones128, rhs=s,
                                             start=(dc == 0), stop=(dc == 3))
                    mu_f = statfp.tile([128, 1024], f32, tag="statf")
                    var_f = statfp.tile([128, 1024], f32, tag="statf")
                    tmp = statfp.tile([128, 1024], f32, tag="statf")
                    nc.vector.tensor_scalar_mul(out=mu_f, in0=mu_ps, scalar1=1.0 / D)
                    nc.vector.tensor_scalar_mul(out=var_f, in0=sq_ps, scalar1=1.0 / D)
                    nc.vector.tensor_mul(out=tmp, in0=mu_f, in1=mu_f)
                    nc.vector.tensor_sub(out=var_f, in0=var_f, in1=tmp)
                    # rstd = exp(-0.5 * ln(var + eps))
                    nc.scalar.activation(out=var_f, in_=var_f, func=AF.Ln,
                                         bias=eps_t[:, :])
                    nc.scalar.activation(out=rstd_bf[:, sl], in_=var_f,
                                         func=AF.Exp, scale=-0.5)
                    nc.vector.tensor_copy(out=mu_bf[:, sl], in_=mu_f)
                return mu_bf, rstd_bf

            def ln_apply(src_tiles, mu_bf, rstd_bf, tlen, tag, pool):
                outs = []
                for dc in range(4):
                    o = pool.tile([128, tlen], bf16, tag=tag)
                    nc.vector.tensor_sub(out=o, in0=src_tiles[dc],
                                         in1=mu_bf[:, 0:tlen])
                    nc.vector.tensor_mul(out=o, in0=o, in1=rstd_bf[:, 0:tlen])
                    outs.append(o)
                return outs

            # ================= stage 1: LN1 -> xnT =================
            mu1, rstd1 = ln_stats(xt, T)
            xnT = ln_apply(xt, mu1, rstd1, T, "sheet", shp)

            # ================= QKV =================
            qT = []
            for jc in range(4):
                q_ps = ps2.tile([128, 1024], f32, tag="ps2")
                for half in range(2):
                    ps_h = slice(half * 512, half * 512 + 512)
                    for dc in range(4):
                        nc.tensor.matmul(
                            q_ps[:, ps_h],
                            lhsT=wq_sb[:, dc, jc * 128:jc * 128 + 128],
                            rhs=xnT[dc][:, half * 512:half * 512 + 512],
                            start=(dc == 0), stop=(dc == 3))
                q_t = qtp.tile([128, 1024], bf16, tag="qt")
                nc.vector.tensor_copy(out=q_t, in_=q_ps)
                qT.append(q_t)

            # V token-major (form 1) — before K so xnT frees for kT slots
            v_sb = []
            for tt in range(16):
                v_ps = ps1.tile([128, 512], f32, tag="ps1")
                for dc in range(4):
                    nc.tensor.matmul(
                        v_ps,
                        lhsT=xnT[dc][:, tt * 128:(tt + 1) * 128],
                        rhs=wv_sb[:, dc, :],
                        start=(dc == 0), stop=(dc == 3))
                v_t = vp.tile([128, 512], bf16, tag="v")
                nc.vector.tensor_copy(out=v_t, in_=v_ps)
                v_sb.append(v_t)

            kT = []
            for jc in range(4):
                k_t = ktp.tile([128, T], bf16, tag="kt")
                for tch in range(2):
                    k_ps = ps2.tile([128, 1024], f32, tag="ps2")
                    for half in range(2):
                        ts0 = tch * 1024 + half * 512
                        ps_h = slice(half * 512, half * 512 + 512)
                        for dc in range(4):
                            nc.tensor.matmul(
                                k_ps[:, ps_h],
                                lhsT=wk_sb[:, dc, jc * 128:jc * 128 + 128],
                                rhs=xnT[dc][:, ts0:ts0 + 512],
                                start=(dc == 0), stop=(dc == 3))
                    nc.vector.tensor_copy(
                        out=k_t[:, tch * 1024:(tch + 1) * 1024], in_=k_ps)
                kT.append(k_t)

            # w2 DMAs now: reuse v-tag slots as they free after attention
            w2_sb = []
            for mc in range(16):
                t = vp.tile([128, 512], bf16, tag="v")
                nc.sync.dma_start(out=t, in_=w2_d[mc * 128:(mc + 1) * 128, :])
                w2_sb.append(t)

            # ================= attention =================
            attn_h = []
            for _h in range(H):
                a_t = attnp.tile([64, 1024], bf16, tag="attn")
                attn_h.append(a_t)
            for h8 in range(H):
                jc = h8 // 2
                rb = (h8 % 2) * 64
                for qc in range(2):
                    o_ps = ps1.tile([64, 512], f32, tag="ps1")
                    z_ps = ps1.tile([64, 512], f32, tag="ps1")
                    for kcp in range(8):
                        s_ps = ps2.tile([128, 1024], f32, tag="ps2")
                        for j in range(2):
                            kc = kcp * 2 + j
                            nc.tensor.matmul(
                                s_ps[:, j * 512:(j + 1) * 512],
                                lhsT=kT[jc][rb:rb + 64, kc * 128:(kc + 1) * 128],
                                rhs=qT[jc][rb:rb + 64, qc * 512:(qc + 1) * 512],
                                start=True, stop=True)
                        e_t = ep.tile([128, 1024], bf16, tag="e")
                        nc.scalar.activation(out=e_t, in_=s_ps, func=AF.Exp,
                                             scale=SCALE)
                        for j in range(2):
                            kc = kcp * 2 + j
                            sl = slice(j * 512, (j + 1) * 512)
                            nc.tensor.matmul(
                                o_ps, lhsT=v_sb[kc][:, h8 * 64:h8 * 64 + 64],
                                rhs=e_t[:, sl],
                                start=(kc == 0), stop=(kc == 15))
                            nc.tensor.matmul(
                                z_ps, lhsT=ones128[:, 0:64], rhs=e_t[:, sl],
                                start=(kc == 0), stop=(kc == 15))
                    z_sb = zrp.tile([64, 512], f32, tag="z")
                    nc.vector.tensor_copy(out=z_sb, in_=z_ps)
                    r_t = zrp.tile([64, 512], f32, tag="r")
                    nc.vector.reciprocal_approx_fast(out=r_t, in_=z_sb)
                    nc.vector.tensor_mul(
                        out=attn_h[h8][:, qc * 512:(qc + 1) * 512],
                        in0=o_ps, in1=r_t)

            # ================= O-projection + residuals =================
            y_sb = []
            for tt in range(8):
                p_ps = ps1.tile([128, 512], f32, tag="ps1")
                for h8 in range(H):
                    nc.tensor.matmul(
                        p_ps, lhsT=attn_h[h8][:, tt * 128:(tt + 1) * 128],
                        rhs=wo_sb[:, h8, :],
                        start=(h8 == 0), stop=(h8 == 7))
                y_t = tokp.tile([128, D], f32, tag="tok")
                nc.vector.tensor_add(out=y_t, in0=p_ps, in1=xtok_sb[tt])
                y_sb.append(y_t)

            yT = []
            for jc in range(4):
                p_ps = ps2.tile([128, 1024], f32, tag="ps2")
                for half in range(2):
                    ps_h = slice(half * 512, half * 512 + 512)
                    for h8 in range(H):
                        nc.tensor.matmul(
                            p_ps[:, ps_h],
                            lhsT=wo_sb[:, h8, jc * 128:jc * 128 + 128],
                            rhs=attn_h[h8][:, half * 512:half * 512 + 512],
                            start=(h8 == 0), stop=(h8 == 7))
                y_t = ytp.tile([128, 1024], bf16, tag="yt")
                nc.vector.tensor_add(out=y_t, in0=p_ps, in1=xt[jc][:, 0:1024])
                yT.append(y_t)

            # ================= LN2 -> ynT =================
            mu2, rstd2 = ln_stats(yT, 1024)
            ynT = ln_apply(yT, mu2, rstd2, 1024, "yt", ytp)

            # ================= MLP =================
            def mlp1(mc):
                h_ps = ps2.tile([128, 1024], f32, tag="ps2")
                for half in range(2):
                    ps_h = slice(half * 512, half * 512 + 512)
                    for jc in range(4):
                        nc.tensor.matmul(
                            h_ps[:, ps_h],
                            lhsT=w1_sb[jc][:, mc * 128:(mc + 1) * 128],
                            rhs=ynT[jc][:, half * 512:half * 512 + 512],
                            start=(jc == 0), stop=(jc == 3))
                h_t = htp.tile([128, 1024], bf16, tag="ht")
                nc.scalar.activation(out=h_t, in_=h_ps, func=AF.Gelu)
                return h_t

            hT = [mlp1(mc) for mc in range(8)]
            # pass A: mc 0..7
            y2 = []
            for tt in range(8):
                o_ps = ps1.tile([128, 512], f32, tag="ps1")
                for mc in range(8):
                    nc.tensor.matmul(
                        o_ps, lhsT=hT[mc][:, tt * 128:(tt + 1) * 128],
                        rhs=w2_sb[mc],
                        start=(mc == 0), stop=(mc == 7))
                y_t = tokp.tile([128, D], f32, tag="tok")
                nc.vector.tensor_add(out=y_t, in0=o_ps, in1=y_sb[tt])
                y2.append(y_t)
            hTb = [mlp1(mc) for mc in range(8, 16)]
            for tt in range(8):
                o_ps = ps1.tile([128, 512], f32, tag="ps1")
                for mc in range(8):
                    nc.tensor.matmul(
                        o_ps, lhsT=hTb[mc][:, tt * 128:(tt + 1) * 128],
                        rhs=w2_sb[8 + mc],
                        start=(mc == 0), stop=(mc == 7))
                y_t = tokp.tile([128, D], f32, tag="tok")
                nc.vector.tensor_add(out=y_t, in0=o_ps, in1=y2[tt])
                nc.sync.dma_start(out=out_d[tt * 128:(tt + 1) * 128, :], in_=y_t)

    nc.compile()
    return nc


def kernel(**inputs):
    global _built
    x = np.asarray(inputs["x"], dtype=np.float32)
    wbf = {n: np.ascontiguousarray(
        np.asarray(inputs[n], dtype=np.float32).astype(ml_dtypes.bfloat16))
        for n in ("Wq", "Wk", "Wv", "Wo", "W1", "W2")}

    if _built is None:
        _built = _build()
    nc = _built

    in_maps = []
    for c in range(8):
        b, hh = c // 2, c % 2
        own = x[b, hh * TOK:(hh + 1) * TOK]
        other = x[b, (1 - hh) * TOK:(2 - hh) * TOK]
        roll = np.concatenate([own, other], axis=0)           # [2048, 512]
        xT = np.ascontiguousarray(roll.T.astype(ml_dtypes.bfloat16))
        in_maps.append({
            "x_tok": np.ascontiguousarray(own),
            "xT": xT,
            "wq": wbf["Wq"], "wk": wbf["Wk"], "wv": wbf["Wv"], "wo": wbf["Wo"],
            "w1": wbf["W1"], "w2": wbf["W2"],
        })

    res = run_bass_kernel_spmd(nc, in_maps, core_ids=list(range(8)))
    out = np.empty((B, T, D), np.float32)
    for c in range(8):
        b, hh = c // 2, c % 2
        out[b, hh * TOK:(hh + 1) * TOK] = res.results[c]["out"]
    return out
